# revision 26
# baseline (speedup 1.0000x reference)
"""DeepseekV2-style MoE block on 8 Trainium2 NeuronCores (Bass/Tile).

Expert-parallel sharding with host-side routing/dispatch. The router
(sigmoid scores, grouped top-2-of-4-groups, top-4 experts, renormalized
combine weights) is a tiny T*E*H fp32 computation; it runs on the host,
which then plays the role of the all-to-all fabric: it gathers each
expert's tokens into a compact, pre-transposed activation block and
scatters the expert outputs back during the final unshard/reduce.

Each core owns 2 routed experts (load-balanced pairing: heaviest with
lightest by token count, so slot capacities are minimal) plus a 1/8
tensor-parallel slice of the shared-expert MLP (intermediate dim).
The device program is a pure bf16 GEMM pipeline - no fp32 matmuls, no
transposes, no gpsimd - ordered to keep the PE in long uninterrupted
streaks (the tensor engine only reaches max clock after ~3us of
continuous execution):

  shared gate_up -> expert-A gate_up -> expert-B gate_up
    -> shared down (dense out) -> expert-A down -> expert-B down (ye)

Weight streams live in tag-separated rings of one persistent pool, so
prefetch for a later phase proceeds while the current one computes and
no phase-transition stalls arise from SBUF address reuse. DMA issue
order is tuned so the first shared gate_up panel lands within ~2us.

Capacities are derived at run time from the actual routing counts
(rounded up to a multiple of 16), so the program adapts to the inputs.

Problem shapes (hardcoded per contract): T=1024, H=2048, E=16, I=1408,
IS=2816, top-4 of 16 with grouped top-2-of-4-groups selection, sigmoid
scoring, renormalized weights, routed scaling 2.5.
"""

import sys

sys.path.insert(0, "/opt/trn_rl_repo")

import numpy as np
import ml_dtypes

import concourse.bass as bass
import concourse.bacc as bacc
import concourse.mybir as mybir
from concourse.tile import TileContext
from concourse.bass_utils import run_bass_kernel_spmd

F32 = mybir.dt.float32
BF16 = mybir.dt.bfloat16
AF = mybir.ActivationFunctionType

T, H, E, I = 1024, 2048, 16, 1408
IS = 2816
N_CORES = 8
E_LOC = E // N_CORES            # 2 routed experts per core
ISL = IS // N_CORES             # 352 shared-intermediate slice per core
ISL_PAD = 384                   # padded to 3x128 (zero-padded cols/rows)
ROUTED_SCALING = 2.5

HC = H // 128                   # 16 h-chunks
IB = (2 * I) // 128             # 22 gate_up column panels per expert
IBH = I // 128                  # 11 (g/u halves)
SB = ISL_PAD // 128             # 3 shared panels per half

N_GROUP = 4
TOPK_GROUP = 2
TOP_K = 4


# --------------------------------------------------------------------------
# host-side router + dispatch planning
# --------------------------------------------------------------------------

def plan_routing(x, gate_w, bias):
    """Replicates the reference router in numpy fp32 and plans the
    expert->core assignment. Returns combine [T,E], per-core expert
    pairs, per-expert token index lists and slot capacities."""
    x = np.asarray(x, np.float32)
    gate_w = np.asarray(gate_w, np.float32)
    bias = np.asarray(bias, np.float32)
    logits = x @ gate_w.T
    scores = 1.0 / (1.0 + np.exp(-logits))
    sb = scores + bias[None, :]
    gs = sb.reshape(T, N_GROUP, E // N_GROUP)
    gsort = np.sort(gs, axis=-1)
    group_score = gsort[..., -1] + gsort[..., -2]
    gidx = np.argsort(-group_score, axis=-1)[:, :TOPK_GROUP]
    gmask = np.zeros((T, N_GROUP), np.float32)
    np.put_along_axis(gmask, gidx, 1.0, axis=1)
    emask = np.repeat(gmask, E // N_GROUP, axis=1)
    masked = np.where(emask > 0, sb, -np.inf)
    topk_ids = np.argsort(-masked, axis=-1)[:, :TOP_K]
    topk_w = np.take_along_axis(scores, topk_ids, axis=1)
    topk_w = topk_w / topk_w.sum(-1, keepdims=True)
    combine = np.zeros((T, E), np.float32)
    np.put_along_axis(combine, topk_ids, topk_w.astype(np.float32), axis=1)

    idx = [np.nonzero(combine[:, e])[0].astype(np.int64) for e in range(E)]
    counts = np.array([len(i) for i in idx])
    order = np.argsort(-counts, kind="stable")
    # heaviest paired with lightest: slot A holds ranks 0..7, slot B 15..8
    pairs = [(int(order[i]), int(order[E - 1 - i])) for i in range(N_CORES)]
    cap_a = int(-(-counts[order[:N_CORES]].max() // 4) * 4)
    cap_b = int(-(-counts[order[N_CORES:]].max() // 4) * 4)
    return {
        "combine": combine, "idx": idx, "counts": counts,
        "pairs": pairs, "cap_a": cap_a, "cap_b": cap_b,
    }


# --------------------------------------------------------------------------
# device program
# --------------------------------------------------------------------------

def _build_program(cap_a, cap_b, sim_compat=False):
    nc = bacc.Bacc()

    xt_b = nc.declare_dram_parameter("xt_b", [128, HC, T], BF16, isOutput=False)
    xe_a = nc.declare_dram_parameter("xe_a", [128, HC, cap_a], BF16, isOutput=False)
    xe_b = nc.declare_dram_parameter("xe_b", [128, HC, cap_b], BF16, isOutput=False)
    ce_a = nc.declare_dram_parameter("ce_a", [128, cap_a], F32, isOutput=False)
    ce_b = nc.declare_dram_parameter("ce_b", [128, cap_b], F32, isOutput=False)
    w_gu = nc.declare_dram_parameter("w_gu", [E_LOC, IB, 128, HC, 128], BF16, isOutput=False)
    w_dn = nc.declare_dram_parameter("w_dn", [E_LOC, 2, IBH, 128, 1024], BF16, isOutput=False)
    s_gu = nc.declare_dram_parameter("s_gu", [2 * SB, 128, HC, 128], BF16, isOutput=False)
    s_dn = nc.declare_dram_parameter("s_dn", [2, SB, 128, 1024], BF16, isOutput=False)
    # outputs are tile-blocked so each PSUM drain is one contiguous DRAM
    # write (large linear packets instead of 1KB strided rows); the host
    # reassembles. out_t: (hh, tg, t, q); ye_t: (l, hh, b, q).
    out_t = nc.declare_dram_parameter("out_t", [2, 4, 2, 2, 128, 512], BF16, isOutput=True)
    nt_a = (cap_a + 127) // 128
    nt_b = (cap_b + 127) // 128
    ye_t = nc.declare_dram_parameter("ye_t", [E_LOC, 2, max(nt_a, nt_b), 2, 128, 512], BF16, isOutput=True)

    caps = (cap_a, cap_b)
    xes = (xe_a, xe_b)
    ces = (ce_a, ce_b)

    with TileContext(nc) as tc:
        with tc.tile_pool(name="resident", bufs=1) as res, \
             tc.tile_pool(name="wp", bufs=8) as wp, \
             tc.tile_pool(name="act", bufs=3) as ap, \
             tc.tile_pool(name="drain", bufs=8) as op:
            # -------- resident tiles + DMA issue order (startup-critical) --
            # Each dma_start lands on ONE of 16 HW queues (~24 GB/s each),
            # so critical transfers are split into small pieces that spread
            # round-robin across queues: latency drops ~Nx.
            xtb = res.tile([128, HC, T], BF16, tag="xtb")
            sgw = [res.tile([128, HC, 128], BF16, tag=f"sgw{p}", name=f"sgw{p}")
                   for p in range(2 * SB)]
            # j=0 panels + x^T chunks interleaved per h-chunk: the PE's
            # c-th accumulation step only needs the c-th pieces.
            for c in range(HC):
                nc.sync.dma_start(out=sgw[0][:, c, :], in_=s_gu[0, :, c, :])
                nc.sync.dma_start(out=sgw[SB][:, c, :], in_=s_gu[SB, :, c, :])
                for h in range(4):
                    sl = slice(h * 256, (h + 1) * 256)
                    nc.sync.dma_start(out=xtb[:, c, sl], in_=xt_b[:, c, sl])
            for p in (1, SB + 1, 2, SB + 2):
                for g in range(4):
                    nc.sync.dma_start(out=sgw[p][:, 4 * g:4 * g + 4, :],
                                      in_=s_gu[p, :, 4 * g:4 * g + 4, :])
            # expert-phase inputs + shared-down weights ride the second
            # HWDGE queue (Activation engine), issued after the first silu
            # so they don't compete with the startup-critical loads.
            xe_sb = [res.tile([128, HC, caps[l]], BF16, tag=f"xe{l}", name=f"xe{l}")
                     for l in range(E_LOC)]
            ce_sb = [res.tile([128, caps[l]], F32, tag=f"ce{l}", name=f"ce{l}")
                     for l in range(E_LOC)]
            sdw = [[res.tile([128, 1024], BF16, tag=f"sdw{hh}_{ic}",
                             name=f"sdw{hh}_{ic}") for ic in range(SB)]
                   for hh in range(2)]

            def issue_expert_loads():
                for l in range(E_LOC):
                    for c in range(HC):
                        nc.scalar.dma_start(out=xe_sb[l][:, c, :],
                                            in_=xes[l][:, c, :])
                    nc.scalar.dma_start(out=ce_sb[l][:], in_=ces[l][:])
                for hh in range(2):
                    for ic in range(SB):
                        for q in range(2):
                            sl = slice(q * 512, (q + 1) * 512)
                            nc.scalar.dma_start(out=sdw[hh][ic][:, sl],
                                                in_=s_dn[hh, ic, :, sl])
            aTs = res.tile([128, SB, T], BF16, tag="aTs")
            aTe = [res.tile([128, IBH, caps[l]], BF16, tag=f"aT{l}", name=f"aTe{l}")
                   for l in range(E_LOC)]

            def silu_into(sg, ps):
                if sim_compat:  # CoreSim has no Silu; silu = x*sigmoid(x)
                    nc.scalar.activation(sg[:], ps[:], AF.Sigmoid)
                    nc.vector.tensor_mul(sg[:], sg[:], ps[:])
                else:
                    nc.scalar.activation(sg[:], ps[:], AF.Silu)

            # ---------------- gate_up section ----------------
            with tc.tile_pool(name="sgu_ps", bufs=2, space="PSUM") as sps, \
                 tc.tile_pool(name="egu_ps", bufs=4, space="PSUM") as eps:
                # shared expert gate_up: full T tokens, resident weights
                for j in range(SB):
                    psg = sps.tile([128, T], F32, tag="ps_sgu", name=f"spsg{j}")
                    psu = sps.tile([128, T], F32, tag="ps_sgu", name=f"spsu{j}")
                    for part, ps in ((j, psg), (j + SB, psu)):
                        for c in range(HC):
                            for th in range(2):
                                sl = slice(th * 512, (th + 1) * 512)
                                nc.tensor.matmul(
                                    ps[:, sl], sgw[part][:, c, :], xtb[:, c, sl],
                                    start=(c == 0), stop=(c == HC - 1))
                    sg = ap.tile([128, T], BF16, tag="silu_g")
                    silu_into(sg, psg)
                    nc.vector.tensor_mul(aTs[:, j, :], sg[:], psu[:])
                    if j == 0:
                        issue_expert_loads()

                # routed expert gate_up: compact tokens, streamed weights
                for l in range(E_LOC):
                    cap = caps[l]
                    for j in range(IBH):
                        psg = eps.tile([128, cap], F32, tag="ps_egu", name=f"epsg{l}_{j}")
                        psu = eps.tile([128, cap], F32, tag="ps_egu", name=f"epsu{l}_{j}")
                        for part, ps in ((j, psg), (j + IBH, psu)):
                            wt = wp.tile([128, HC, 128], BF16, tag="wgu",
                                         name=f"wt{l}_{part}")
                            # alternate HWDGE queues for aggregate bandwidth,
                            # 4 pieces per panel for low arrival latency
                            eng = nc.sync if part % 2 == 0 else nc.scalar
                            for g in range(4):
                                eng.dma_start(
                                    out=wt[:, 4 * g:4 * g + 4, :],
                                    in_=w_gu[l, part, :, 4 * g:4 * g + 4, :])
                            for c in range(HC):
                                nc.tensor.matmul(
                                    ps[:], wt[:, c, :], xe_sb[l][:, c, :],
                                    start=(c == 0), stop=(c == HC - 1))
                        sg = ap.tile([128, cap], BF16, tag="silu_g")
                        silu_into(sg, psg)
                        su = ap.tile([128, cap], BF16, tag="su")
                        nc.vector.tensor_mul(su[:], sg[:], psu[:])
                        nc.vector.tensor_mul(aTe[l][:, j, :], su[:], ce_sb[l][:])

            # ---------------- down section ----------------
            # [128,512] PSUM tiles (1 bank each), one deep ring shared by
            # both down phases; drains alternate scalar/vector engines.
            def drain(dst_ap, ps, rows, eng):
                ot = op.tile([128, 512], BF16, tag="ot")
                if eng == 0:
                    nc.scalar.copy(ot[:rows, :], ps[:rows, :])
                else:
                    nc.vector.tensor_copy(ot[:rows, :], ps[:rows, :])
                r0 = min(rows, 64)
                nc.scalar.dma_start(out=dst_ap[0:r0, :], in_=ot[0:r0, :])
                if rows > 64:
                    nc.scalar.dma_start(out=dst_ap[64:rows, :],
                                        in_=ot[64:rows, :])

            with tc.tile_pool(name="dn_ps", bufs=8, space="PSUM") as dps:
                # shared expert down: dense [T, H] into `out`
                for hh in range(2):
                    for tg in range(4):   # groups of 2 token tiles
                        ts0 = tg * 2
                        psd = [[dps.tile([128, 512], F32, tag="ps_dn",
                                         name=f"psds{hh}_{ts0 + t}_{q}")
                                for q in range(2)] for t in range(2)]
                        for ic in range(SB):
                            for t in range(2):
                                for q in range(2):
                                    nc.tensor.matmul(
                                        psd[t][q][:],
                                        aTs[:, ic, (ts0 + t) * 128:(ts0 + t + 1) * 128],
                                        sdw[hh][ic][:, q * 512:(q + 1) * 512],
                                        start=(ic == 0), stop=(ic == SB - 1))
                        for t in range(2):
                            for q in range(2):
                                drain(out_t[hh, tg, t, q], psd[t][q], 128,
                                      (t * 2 + q) % 2)

                # routed expert down: compact token tiles into `ye_t`
                for l in range(E_LOC):
                    cap = caps[l]
                    ntile = (cap + 127) // 128
                    for hh in range(2):
                        psd = [[dps.tile([128, 512], F32, tag="ps_dn",
                                         name=f"psde{l}_{hh}_{b}_{q}")
                                for q in range(2)] for b in range(ntile)]
                        for ic in range(IBH):
                            wd = wp.tile([128, 1024], BF16, tag="wd",
                                         name=f"ewd{l}_{hh}_{ic}")
                            for q in range(2):
                                sl = slice(q * 512, (q + 1) * 512)
                                nc.sync.dma_start(out=wd[:, sl],
                                                  in_=w_dn[l, hh, ic, :, sl])
                            for b in range(ntile):
                                rows = min(128, cap - b * 128)
                                for q in range(2):
                                    nc.tensor.matmul(
                                        psd[b][q][:rows, :],
                                        aTe[l][:, ic, b * 128:b * 128 + rows],
                                        wd[:, q * 512:(q + 1) * 512],
                                        start=(ic == 0), stop=(ic == IBH - 1))
                        for b in range(ntile):
                            rows = min(128, cap - b * 128)
                            for q in range(2):
                                drain(ye_t[l, hh, b, q, 0:rows, :],
                                      psd[b][q], rows, (b + q) % 2)
    nc.compile()
    return nc


_PROGRAM = {}


def _get_program(cap_a, cap_b, sim_compat=False):
    key = (cap_a, cap_b, sim_compat)
    if key not in _PROGRAM:
        _PROGRAM[key] = _build_program(cap_a, cap_b, sim_compat)
    return _PROGRAM[key]


# --------------------------------------------------------------------------
# host-side input packing
# --------------------------------------------------------------------------

def make_in_maps(plan, hidden_states, w_gate_up, w_down,
                 shared_gate_up, shared_down):
    x = np.asarray(hidden_states, np.float32)
    xb = x.astype(ml_dtypes.bfloat16)
    # partition-major [128, HC, T] so the resident load is contiguous per chunk
    xt_b = np.ascontiguousarray(
        xb.T.reshape(HC, 128, T).transpose(1, 0, 2))

    wgu = np.asarray(w_gate_up, np.float32).astype(ml_dtypes.bfloat16)  # [E,H,2I]
    wdn = np.asarray(w_down, np.float32).astype(ml_dtypes.bfloat16)    # [E,I,H]
    sgu = np.asarray(shared_gate_up, np.float32).astype(ml_dtypes.bfloat16)
    sdn = np.asarray(shared_down, np.float32).astype(ml_dtypes.bfloat16)

    combine = plan["combine"]
    caps = (plan["cap_a"], plan["cap_b"])

    in_maps = []
    for c in range(N_CORES):
        m = {"xt_b": xt_b}
        experts = plan["pairs"][c]
        # routed experts' weights, panelized
        wg = wgu[list(experts)]                        # [2, H, 2I]
        m["w_gu"] = np.ascontiguousarray(
            wg.reshape(E_LOC, HC, 128, IB, 128)
              .transpose(0, 3, 2, 1, 4))               # [2, IB, 128, HC, 128]
        wd = wdn[list(experts)]                        # [2, I, H]
        m["w_dn"] = np.ascontiguousarray(
            wd.reshape(E_LOC, IBH, 128, 2, 1024).transpose(0, 3, 1, 2, 4))
        # compact token blocks + combine rows per slot
        for l, name in enumerate(("a", "b")):
            e = experts[l]
            idx = plan["idx"][e]
            n = len(idx)
            cap = caps[l]
            xe = np.zeros((cap, H), ml_dtypes.bfloat16)
            xe[:n] = xb[idx]
            m[f"xe_{name}"] = np.ascontiguousarray(
                xe.T.reshape(HC, 128, cap).transpose(1, 0, 2))
            ce = np.zeros((cap,), np.float32)
            ce[:n] = combine[idx, e] * ROUTED_SCALING
            m[f"ce_{name}"] = np.ascontiguousarray(
                np.broadcast_to(ce, (128, cap)))
        # shared slice: g cols [c*ISL, (c+1)*ISL), u cols IS + same, pad to 384
        g_sl = sgu[:, ISL * c:ISL * (c + 1)]
        u_sl = sgu[:, IS + ISL * c:IS + ISL * (c + 1)]
        pad = np.zeros((H, ISL_PAD - ISL), ml_dtypes.bfloat16)
        s_gu_c = np.concatenate([g_sl, pad, u_sl, pad], axis=1)    # [H, 2*384]
        m["s_gu"] = np.ascontiguousarray(
            s_gu_c.reshape(HC, 128, 2 * SB, 128)
                  .transpose(2, 1, 0, 3))               # [6, 128, HC, 128]
        d_sl = sdn[ISL * c:ISL * (c + 1)]                          # [ISL, H]
        d_pad = np.concatenate(
            [d_sl, np.zeros((ISL_PAD - ISL, H), ml_dtypes.bfloat16)], axis=0)
        m["s_dn"] = np.ascontiguousarray(
            d_pad.reshape(SB, 128, 2, 1024).transpose(2, 0, 1, 3))  # [2, 3, 128, 1024]
        in_maps.append(m)
    return in_maps


def kernel(hidden_states, gate_w, bias, w_gate_up, w_down,
           shared_gate_up, shared_down, num_global_tokens=None,
           max_num_tokens_per_gpu=None, **_unused):
    plan = plan_routing(hidden_states, gate_w, bias)
    nc = _get_program(plan["cap_a"], plan["cap_b"])
    in_maps = make_in_maps(plan, hidden_states, w_gate_up, w_down,
                           shared_gate_up, shared_down)
    res = run_bass_kernel_spmd(nc, in_maps, list(range(N_CORES)))
    acc = np.zeros((T, H), np.float64)
    caps = (plan["cap_a"], plan["cap_b"])
    for c in range(N_CORES):
        # out_t [hh, tg, t, q, 128, 512] -> dense [T, H]
        ot = np.asarray(res.results[c]["out_t"], np.float64)
        acc += (ot.transpose(1, 2, 4, 0, 3, 5)        # tg, t, 128, hh, q, 512
                  .reshape(T, H))
        yec = np.asarray(res.results[c]["ye_t"], np.float64)
        for l in range(E_LOC):
            e = plan["pairs"][c][l]
            idx = plan["idx"][e]
            # ye_t [hh, b, q, 128, 512] -> [ntile*128, H]
            y = (yec[l].transpose(1, 3, 0, 2, 4)      # b, 128, hh, q, 512
                       .reshape(-1, H))
            acc[idx] += y[:len(idx)]
    return acc.astype(np.float32)
